# revision 2
# baseline (speedup 1.0000x reference)
# Trainium2 Bass kernel for nn_CustomImageCosineSimLoss (N=4096, D=512, 8 cores).
#
# Strategy (sharding_hint): shard image rows across the 8 cores (data parallel
# over i); text features / instruction ids are replicated. Each core computes
# its [512, 4096] block of both pairwise matrices and a scalar partial; the
# host sums the 8 partials (the "all-reduce") and divides by N^2.
#
# Math per core (L=512 local rows, G=64 instruction groups):
#   loss*N^2 (core part) = sum_ij relu(cos_ij - 8*mask_ij - w_ij) + G1 - maskcos
# where w_ij = (sim_ij - mn_i) * invr_i with per-row min/max of the raw
# text-text similarity sim, cos is the image/text cosine (row-normalized
# operands), mask_ij = [instr_i == instr_j].  The -8*mask term (folded into
# the cos PSUM accumulation as a one-hot matmul) forces relu() to 0 on
# aligned pairs, whose exact contribution sum_aligned (1 - cos) is computed
# separately via group-sum matmuls (G1 = #aligned pairs, maskcos =
# sum_g <sum_{i in g} ihat_i, sum_{j in g} that_j>).
#
# Engine mapping per [128, 512] tile: PE does sim / cos+mask matmuls (bf16
# operands, fp32 PSUM), ACT copies sim PSUM->SBUF (bf16), DVE does min/max
# stats and the fused x = sim*invr - cos'' pass, ACT does relu(-x + mn*invr)
# with per-row accumulation.  Text/image row normalization (norms on DVE,
# scales on GPSIMD) and DMA-transposes build the d-major operands on chip.
import numpy as np
import ml_dtypes

import concourse.mybir as mybir
import concourse.tile as tile
from concourse import bacc
from concourse.bass import ts

BF16 = mybir.dt.bfloat16
F32 = mybir.dt.float32
AF = mybir.ActivationFunctionType
OP = mybir.AluOpType
nbf = ml_dtypes.bfloat16

N, D, G, NCORES = 4096, 512, 64, 8
L = N // NCORES            # 512 local rows per core
KT = D // 128              # 4 contraction chunks
IT = L // 128              # 4 local i-tiles
JT = N // 512              # 8 j-tiles
TCH = N // 128             # 32 text row chunks
GRP = TCH // IT            # text chunks prepped per i-tile group
BIG = 8.0
EPS_W = 1e-6

_CACHE = {}


def _build_program():
    nc = bacc.Bacc("TRN2", target_bir_lowering=False, debug=False,
                   enable_asserts=True, num_devices=NCORES)

    d_txt_T = nc.dram_tensor("txt_T", [D, N], BF16, kind="ExternalInput").ap()
    d_txt_T_loc = nc.dram_tensor("txt_T_loc", [D, L], BF16, kind="ExternalInput").ap()
    d_txt_rows = nc.dram_tensor("txt_rows", [N, D], BF16, kind="ExternalInput").ap()
    d_img_rows = nc.dram_tensor("img_rows", [L, D], BF16, kind="ExternalInput").ap()
    d_oh_scaled = nc.dram_tensor("oh_scaled", [G, L], BF16, kind="ExternalInput").ap()
    d_oh_rhsT = nc.dram_tensor("oh_rhsT", [G, N], BF16, kind="ExternalInput").ap()
    d_oh_iT = nc.dram_tensor("oh_iT", [L, G], BF16, kind="ExternalInput").ap()
    d_oh_jT = nc.dram_tensor("oh_jT", [N, G], BF16, kind="ExternalInput").ap()
    d_partials = nc.dram_tensor("partials", [128, 8], F32, kind="ExternalOutput").ap()

    with tile.TileContext(nc) as tc:
        with (
            tc.tile_pool(name="persist", bufs=1) as pp,
            tc.tile_pool(name="rows", bufs=GRP + 1) as prow,
            tc.tile_pool(name="sims", bufs=IT) as psim,
            tc.tile_pool(name="main", bufs=2) as pm,
            tc.tile_pool(name="small", bufs=1) as psm,
            tc.tile_pool(name="stats", bufs=2) as pst,
            tc.tile_pool(name="psum", bufs=4, space="PSUM") as pps,
            tc.tile_pool(name="psum1", bufs=2, space="PSUM") as pps1,
        ):
            txt_T_loc = pp.tile([128, KT * L], BF16)
            nc.sync.dma_start(txt_T_loc[:].rearrange("p (c i) -> p c i", c=KT),
                              d_txt_T_loc.rearrange("(c p) i -> p c i", p=128))
            txt_T_loc_v = txt_T_loc[:].rearrange("p (c i) -> p c i", c=KT)

            img_rows = pp.tile([128, IT * D], BF16)
            nc.sync.dma_start(img_rows[:].rearrange("p (t d) -> p t d", t=IT),
                              d_img_rows.rearrange("(t p) d -> p t d", p=128))
            img_rows_v = img_rows[:].rearrange("p (t d) -> p t d", t=IT)

            oh_scaled = pp.tile([G, L], BF16)
            nc.sync.dma_start(oh_scaled[:], d_oh_scaled)
            oh_rhsT = pp.tile([G, N], BF16)
            nc.sync.dma_start(oh_rhsT[:], d_oh_rhsT)
            oh_iT = pp.tile([128, IT * G], BF16)
            nc.sync.dma_start(oh_iT[:].rearrange("p (t g) -> p t g", t=IT),
                              d_oh_iT.rearrange("(t p) g -> p t g", p=128))
            oh_iT_v = oh_iT[:].rearrange("p (t g) -> p t g", t=IT)
            oh_jT = pp.tile([128, TCH * G], BF16)
            nc.sync.dma_start(oh_jT[:].rearrange("p (t g) -> p t g", t=TCH),
                              d_oh_jT.rearrange("(t p) g -> p t g", p=128))
            oh_jT_v = oh_jT[:].rearrange("p (t g) -> p t g", t=TCH)

            # txt_T loaded per j-block so the first sim matmul starts early
            txt_T = pp.tile([128, KT * N], BF16)
            txt_T_v = txt_T[:].rearrange("p (c j) -> p c j", c=KT)
            d_txt_T_v = d_txt_T.rearrange("(c p) j -> p c j", p=128)
            for jt in range(JT):
                nc.sync.dma_start(txt_T_v[:, :, ts(jt, 512)],
                                  d_txt_T_v[:, :, ts(jt, 512)])

            that_T = pp.tile([128, KT * N], BF16)
            that_T_v = that_T[:].rearrange("p (c j) -> p c j", c=KT)
            ihat_T = pp.tile([128, KT * L], BF16)
            ihat_T_v = ihat_T[:].rearrange("p (c i) -> p c i", c=KT)

            # ---------- image prep ----------
            nsq_i = psm.tile([128, IT], F32)
            ihat_rows = pp.tile([128, IT * D], BF16)
            ihat_rows_v = ihat_rows[:].rearrange("p (t d) -> p t d", t=IT)
            for t in range(IT):
                junk = pst.tile([128, D], BF16, tag="junk")
                nc.vector.scalar_tensor_tensor(
                    out=junk[:], in0=img_rows_v[:, t, :], scalar=1.0,
                    in1=img_rows_v[:, t, :], op0=OP.mult, op1=OP.mult,
                    accum_out=nsq_i[:, t:t + 1])
            n_i = psm.tile([128, IT], F32)
            nc.scalar.sqrt(n_i[:], nsq_i[:])
            inv_ni = psm.tile([128, IT], F32)
            nc.vector.reciprocal(inv_ni[:], n_i[:])
            for t in range(IT):
                nc.gpsimd.tensor_scalar_mul(out=ihat_rows_v[:, t, :],
                                            in0=img_rows_v[:, t, :],
                                            scalar1=inv_ni[:, t:t + 1])
            for t in range(IT):  # [i,d] -> [d,i] via DMA xbar
                nc.sync.dma_start(out=ihat_T_v[:, :, ts(t, 128)],
                                  in_=ihat_rows_v[:, t, :], transpose=True)

            # ---- interleaved: sim sweep(it) + text prep group(it) ----
            nsq_t = psm.tile([128, TCH], F32)
            n_t = psm.tile([128, TCH], F32)
            inv_nt = psm.tile([128, TCH], F32)
            psum_TXT = pps1.tile([G, D], F32, tag="txt")
            comb = psm.tile([128, 8], F32)
            nc.gpsimd.memset(comb[:], 0.0)

            sim_sbs, invrs, mninvrs = [], [], []
            for it in range(IT):
                sim_sb = psim.tile([128, N], BF16, tag="sim")
                for jt in range(JT):
                    ps = pps.tile([128, 512], F32, tag="mm")
                    for kt in range(KT):
                        nc.tensor.matmul(ps[:], txt_T_loc_v[:, kt, ts(it, 128)],
                                         txt_T_v[:, kt, ts(jt, 512)],
                                         start=(kt == 0), stop=(kt == KT - 1))
                    nc.scalar.copy(sim_sb[:, ts(jt, 512)], ps[:])

                g0 = it * GRP
                tr_tiles = []
                for t in range(g0, g0 + GRP):
                    tr = prow.tile([128, D], BF16, tag="txtrows")
                    nc.sync.dma_start(tr[:], d_txt_rows[ts(t, 128), :])
                    tr_tiles.append(tr)
                    junk = pst.tile([128, D], BF16, tag="junk")
                    nc.vector.scalar_tensor_tensor(
                        out=junk[:], in0=tr[:], scalar=1.0,
                        in1=tr[:], op0=OP.mult, op1=OP.mult,
                        accum_out=nsq_t[:, t:t + 1])
                nc.scalar.sqrt(n_t[:, g0:g0 + GRP], nsq_t[:, g0:g0 + GRP])
                nc.vector.reciprocal(inv_nt[:, g0:g0 + GRP], n_t[:, g0:g0 + GRP])
                for t in range(g0, g0 + GRP):
                    th = prow.tile([128, D], BF16, tag="thatrows")
                    nc.gpsimd.tensor_scalar_mul(out=th[:], in0=tr_tiles[t - g0][:],
                                                scalar1=inv_nt[:, t:t + 1])
                    nc.tensor.matmul(psum_TXT[:], oh_jT_v[:, t, :], th[:],
                                     start=(t == 0), stop=(t == TCH - 1))
                    nc.sync.dma_start(out=that_T_v[:, :, ts(t, 128)],
                                      in_=th[:], transpose=True)

                mn = pst.tile([128, 1], F32, tag="mn")
                nc.vector.tensor_reduce(out=mn[:], in_=sim_sb[:],
                                        axis=mybir.AxisListType.X, op=OP.min)
                mx = pst.tile([128, 1], F32, tag="mx")
                nc.vector.tensor_reduce(out=mx[:], in_=sim_sb[:],
                                        axis=mybir.AxisListType.X, op=OP.max)
                invr = pst.tile([128, 1], F32, tag="invr")
                rng = pst.tile([128, 1], F32, tag="rng")
                nc.vector.tensor_tensor(out=rng[:], in0=mx[:], in1=mn[:],
                                        op=OP.subtract)
                nc.vector.tensor_scalar_add(out=rng[:], in0=rng[:], scalar1=EPS_W)
                nc.vector.reciprocal(invr[:], rng[:])
                mninvr = pst.tile([128, 1], F32, tag="mninvr")
                nc.vector.tensor_tensor(out=mninvr[:], in0=mn[:], in1=invr[:],
                                        op=OP.mult)
                sim_sbs.append(sim_sb); invrs.append(invr); mninvrs.append(mninvr)

            # ---------- group-sum terms ----------
            psum_IMG = pps1.tile([G, D], F32, tag="img")
            for t in range(IT):
                nc.tensor.matmul(psum_IMG[:], oh_iT_v[:, t, :], ihat_rows_v[:, t, :],
                                 start=(t == 0), stop=(t == IT - 1))
            IMG_s = psm.tile([G, D], F32)
            nc.scalar.copy(IMG_s[:], psum_IMG[:])
            junk2 = psm.tile([G, D], F32)
            nc.vector.scalar_tensor_tensor(
                out=junk2[:], in0=IMG_s[:], scalar=1.0,
                in1=psum_TXT[:], op0=OP.mult, op1=OP.mult,
                accum_out=comb[0:G, 5:6])
            ngl = psm.tile([G, 1], F32)   # = -8 * ng_local
            nc.vector.tensor_reduce(out=ngl[:], in_=oh_scaled[:],
                                    axis=mybir.AxisListType.X, op=OP.add)
            ngg = psm.tile([G, 1], F32)
            nc.vector.tensor_reduce(out=ngg[:], in_=oh_rhsT[:],
                                    axis=mybir.AxisListType.X, op=OP.add)
            junk3 = psm.tile([G, 1], F32)
            nc.vector.scalar_tensor_tensor(
                out=junk3[:], in0=ngl[:], scalar=-1.0 / BIG,
                in1=ngg[:], op0=OP.mult, op1=OP.mult,
                accum_out=comb[0:G, 4:5])

            # ---------- sweep 2: cos + mask, x-pass, relu accumulate ----------
            for it in range(IT):
                sim_sb, invr, mninvr = sim_sbs[it], invrs[it], mninvrs[it]
                x_sb = pm.tile([128, N], BF16, tag="x")
                for jt in range(JT):
                    pc = pps.tile([128, 512], F32, tag="mm")
                    for kt in range(KT):
                        nc.tensor.matmul(pc[:], ihat_T_v[:, kt, ts(it, 128)],
                                         that_T_v[:, kt, ts(jt, 512)],
                                         start=(kt == 0), stop=False)
                    nc.tensor.matmul(pc[:], oh_scaled[:, ts(it, 128)],
                                     oh_rhsT[:, ts(jt, 512)],
                                     start=False, stop=True)
                    nc.vector.scalar_tensor_tensor(
                        out=x_sb[:, ts(jt, 512)], in0=sim_sb[:, ts(jt, 512)],
                        scalar=invr[:], in1=pc[:],
                        op0=OP.mult, op1=OP.subtract)
                rscr = pm.tile([128, N], BF16, tag="rscr")
                nc.scalar.activation(
                    out=rscr[:], in_=x_sb[:], func=AF.Relu,
                    bias=mninvr[:], scale=-1.0,
                    accum_out=comb[:, it:it + 1])

            nc.sync.dma_start(d_partials, comb[:])

    nc.compile()
    return nc


def _host_in_maps(image_features, text_features, instr_d):
    img = np.asarray(image_features, np.float32)
    txt = np.asarray(text_features, np.float32)
    ins = np.asarray(instr_d)
    oh = (ins[None, :] == np.arange(G, dtype=ins.dtype)[:, None]).astype(np.float32)

    txt_b = txt.astype(nbf)
    txt_T_b = np.ascontiguousarray(txt.T).astype(nbf)
    oh_rhsT_b = oh.astype(nbf)
    oh_jT_b = np.ascontiguousarray(oh.T).astype(nbf)

    in_maps = []
    for c in range(NCORES):
        sl = slice(c * L, (c + 1) * L)
        in_maps.append({
            "txt_T": txt_T_b,
            "txt_T_loc": np.ascontiguousarray(txt_T_b[:, sl]),
            "txt_rows": txt_b,
            "img_rows": img[sl].astype(nbf),
            "oh_scaled": np.ascontiguousarray(-BIG * oh[:, sl]).astype(nbf),
            "oh_rhsT": oh_rhsT_b,
            "oh_iT": np.ascontiguousarray(oh_jT_b[sl]),
            "oh_jT": oh_jT_b,
        })
    return in_maps


def kernel(**inputs) -> np.ndarray:
    from concourse.bass_utils import run_bass_kernel_spmd

    if "nc" not in _CACHE:
        _CACHE["nc"] = _build_program()
    nc = _CACHE["nc"]
    in_maps = _host_in_maps(**inputs)
    res = run_bass_kernel_spmd(nc, in_maps, core_ids=list(range(NCORES)),
                               trace=False)
    _CACHE["last_results"] = res
    total = np.float64(0.0)
    for r in res.results:
        p = np.asarray(r["partials"], np.float64)
        total += p[:, 0:5].sum() - p[:, 5].sum() + p[:, 6:].sum()
    return np.float32(total / (N * N))



# revision 3
# speedup vs baseline: 3.2032x; 3.2032x over previous
# Trainium2 Bass kernel for nn_CustomImageCosineSimLoss (N=4096, D=512, 8 cores).
#
# Strategy (sharding_hint): shard image rows across the 8 cores (data parallel
# over i); text features / instruction ids replicated. Each core computes its
# [512, 4096] block of both pairwise matrices and 32 partial sums; the host
# adds the 8*32 partials plus a closed-form aligned-pair correction and
# divides by N^2 (the "all-reduce").
#
# Math per core (L=512 local rows, G=64 instruction groups):
#   device part = sum_ij relu(cos_ij - 8*mask_ij - w_ij)
# where sim'_ij = that_i . t_j (= sim_ij / n_i, so the min-max weights
# w_ij = (sim'_ij - mn'_i) * invr'_i match the reference up to an O(1e-7)
# epsilon shift), cos_ij = ihat_i . that_j, and the -8*mask one-hot matmul
# (folded into the cos PSUM group) forces relu() = 0 on aligned pairs.
# The exact aligned contribution sum_aligned (1 - cos) is computed on the
# host in fp64 from group sums (O(N*D) work).
#
# Device mapping per [128, 512] tile: PE does the sim'/cos matmuls in fp8
# DoubleRow mode (2x rate, fp32 PSUM) plus the plain-fp8 mask matmul; ACT
# copies sim' PSUM->SBUF (bf16); DVE does min/max stats, the fused
# x = pc - sim'*invr' pass, and relu(x + mn'*invr') with per-row
# accumulation. All operands arrive pre-normalized/transposed from the host
# (text is replicated, so normalization is O(N*D) host work), which keeps
# GPSIMD and the DMA-transpose path entirely out of the kernel.
import numpy as np
import ml_dtypes

import concourse.mybir as mybir
import concourse.tile as tile
from concourse import bacc
from concourse.bass import ts

BF16 = mybir.dt.bfloat16
F32 = mybir.dt.float32
FP8 = mybir.dt.float8e4
AF = mybir.ActivationFunctionType
OP = mybir.AluOpType
PM = mybir.MatmulPerfMode
nf8 = ml_dtypes.float8_e4m3

N, D, G, NCORES = 4096, 512, 64, 8
L = N // NCORES            # 512 local rows per core
KT = D // 128              # 4 contraction subtiles
KP = KT // 2               # 2 DoubleRow pairs
IT = L // 128              # 4 local i-tiles
JT = N // 512              # 8 j-blocks
BIG = 8.0
EPS_W = 1e-6

_CACHE = {}


def _build_program():
    nc = bacc.Bacc("TRN2", target_bir_lowering=False, debug=False,
                   enable_asserts=True, num_devices=NCORES)

    d_txt_T = nc.dram_tensor("txt_T", [D, N], FP8, kind="ExternalInput").ap()
    d_that_T = nc.dram_tensor("that_T", [D, N], FP8, kind="ExternalInput").ap()
    d_that_loc = nc.dram_tensor("that_T_loc", [D, L], FP8, kind="ExternalInput").ap()
    d_ihat_loc = nc.dram_tensor("ihat_T_loc", [D, L], FP8, kind="ExternalInput").ap()
    d_oh_scaled = nc.dram_tensor("oh_scaled", [G, L], FP8, kind="ExternalInput").ap()
    d_oh_rhsT = nc.dram_tensor("oh_rhsT", [G, N], FP8, kind="ExternalInput").ap()
    d_partials = nc.dram_tensor("partials", [128, IT * JT], F32,
                                kind="ExternalOutput").ap()

    with tile.TileContext(nc) as tc:
        with (
            tc.tile_pool(name="persist", bufs=1) as pp,
            tc.tile_pool(name="sims", bufs=IT) as psim,
            tc.tile_pool(name="x", bufs=3) as px,
            tc.tile_pool(name="junk", bufs=3) as pj,
            tc.tile_pool(name="stats", bufs=2) as pst,
            tc.tile_pool(name="psA", bufs=3, space="PSUM") as ppsA,
            tc.tile_pool(name="psB", bufs=3, space="PSUM") as ppsB,
        ):
            that_loc = pp.tile([128, KT * L], FP8)
            nc.sync.dma_start(that_loc[:].rearrange("p (c i) -> p c i", c=KT),
                              d_that_loc.rearrange("(c p) i -> p c i", p=128))
            that_loc_v = that_loc[:].rearrange("p (c i) -> p c i", c=KT)

            # txt_T loaded per j-block so the first sim' matmul starts early
            txt_T = pp.tile([128, KT * N], FP8)
            txt_T_v = txt_T[:].rearrange("p (c j) -> p c j", c=KT)
            d_txt_T_v = d_txt_T.rearrange("(c p) j -> p c j", p=128)
            for jt in range(JT):
                nc.sync.dma_start(txt_T_v[:, :, ts(jt, 512)],
                                  d_txt_T_v[:, :, ts(jt, 512)])

            ihat_loc = pp.tile([128, KT * L], FP8)
            nc.sync.dma_start(ihat_loc[:].rearrange("p (c i) -> p c i", c=KT),
                              d_ihat_loc.rearrange("(c p) i -> p c i", p=128))
            ihat_loc_v = ihat_loc[:].rearrange("p (c i) -> p c i", c=KT)
            oh_scaled = pp.tile([G, L], FP8)
            nc.sync.dma_start(oh_scaled[:], d_oh_scaled)
            oh_rhsT = pp.tile([G, N], FP8)
            nc.sync.dma_start(oh_rhsT[:], d_oh_rhsT)

            that_T = pp.tile([128, KT * N], FP8)
            that_T_v = that_T[:].rearrange("p (c j) -> p c j", c=KT)
            d_that_T_v = d_that_T.rearrange("(c p) j -> p c j", p=128)
            for jt in range(JT):
                nc.sync.dma_start(that_T_v[:, :, ts(jt, 512)],
                                  d_that_T_v[:, :, ts(jt, 512)])

            comb = pp.tile([128, IT * JT], F32)
            sims, stats = {}, {}

            def emit_sim(it):
                sim_sb = psim.tile([128, N], BF16, tag="sim")
                for jt in range(JT):
                    ps = ppsA.tile([128, 512], F32, tag="mmA")
                    for kp in range(KP):
                        nc.tensor.matmul(ps[:],
                                         that_loc_v[:, 2 * kp:2 * kp + 2, ts(it, 128)],
                                         txt_T_v[:, 2 * kp:2 * kp + 2, ts(jt, 512)],
                                         start=(kp == 0), stop=(kp == KP - 1),
                                         perf_mode=PM.DoubleRow)
                    nc.scalar.copy(sim_sb[:, ts(jt, 512)], ps[:])
                sims[it] = sim_sb

            def emit_stats(it):
                sim_sb = sims[it]
                mn = pst.tile([128, 1], F32, tag="mn")
                nc.vector.tensor_reduce(out=mn[:], in_=sim_sb[:],
                                        axis=mybir.AxisListType.X, op=OP.min)
                mx = pst.tile([128, 1], F32, tag="mx")
                nc.vector.tensor_reduce(out=mx[:], in_=sim_sb[:],
                                        axis=mybir.AxisListType.X, op=OP.max)
                rng = pst.tile([128, 1], F32, tag="rng")
                nc.vector.tensor_tensor(out=rng[:], in0=mx[:], in1=mn[:],
                                        op=OP.subtract)
                nc.vector.tensor_scalar_add(out=rng[:], in0=rng[:], scalar1=EPS_W)
                invr = pst.tile([128, 1], F32, tag="invr")
                nc.vector.reciprocal(invr[:], rng[:])
                ninvr = pst.tile([128, 1], F32, tag="ninvr")
                nc.vector.tensor_scalar_mul(out=ninvr[:], in0=invr[:], scalar1=-1.0)
                mninvr = pst.tile([128, 1], F32, tag="mninvr")
                nc.vector.tensor_tensor(out=mninvr[:], in0=mn[:], in1=invr[:],
                                        op=OP.mult)
                stats[it] = (ninvr, mninvr)

            def emit_cos(it):
                sim_sb = sims[it]
                ninvr, mninvr = stats[it]
                for jt in range(JT):
                    pc = ppsB.tile([128, 512], F32, tag="mmB")
                    for kp in range(KP):
                        nc.tensor.matmul(pc[:],
                                         ihat_loc_v[:, 2 * kp:2 * kp + 2, ts(it, 128)],
                                         that_T_v[:, 2 * kp:2 * kp + 2, ts(jt, 512)],
                                         start=(kp == 0), stop=False,
                                         perf_mode=PM.DoubleRow)
                    nc.tensor.matmul(pc[:], oh_scaled[:, ts(it, 128)],
                                     oh_rhsT[:, ts(jt, 512)],
                                     start=False, stop=True)
                    # x = pc - sim' * invr'   (DVE, reads PSUM once)
                    x = px.tile([128, 512], BF16, tag="x")
                    nc.vector.scalar_tensor_tensor(
                        out=x[:], in0=sim_sb[:, ts(jt, 512)], scalar=ninvr[:],
                        in1=pc[:], op0=OP.mult, op1=OP.add)
                    # relu(x + mn'*invr') with per-row accumulation (DVE)
                    junk = pj.tile([128, 512], BF16, tag="junk")
                    nc.vector.tensor_scalar(
                        out=junk[:], in0=x[:], scalar1=mninvr[:], scalar2=0.0,
                        op0=OP.add, op1=OP.max,
                        accum_out=comb[:, it * JT + jt:it * JT + jt + 1])

            # software pipeline: PE order sim0 sim1 cos0 sim2 cos1 sim3 cos2 cos3
            emit_sim(0)
            emit_stats(0)
            emit_sim(1)
            emit_stats(1)
            emit_cos(0)
            emit_sim(2)
            emit_stats(2)
            emit_cos(1)
            emit_sim(3)
            emit_stats(3)
            emit_cos(2)
            emit_cos(3)

            nc.sync.dma_start(d_partials, comb[:])

    nc.compile()
    return nc


def _host_prep(image_features, text_features, instr_d):
    img = np.asarray(image_features, np.float64)
    txt = np.asarray(text_features, np.float64)
    ins = np.asarray(instr_d).astype(np.int64)

    nt = np.linalg.norm(txt, axis=1)
    ni = np.linalg.norm(img, axis=1)
    that = txt / nt[:, None]
    ihat = img / ni[:, None]

    txt_T8 = np.ascontiguousarray(txt.T.astype(np.float32)).astype(nf8)
    that_T8 = np.ascontiguousarray(that.T.astype(np.float32)).astype(nf8)
    oh = ins[None, :] == np.arange(G, dtype=np.int64)[:, None]   # [G, N]
    oh_rhsT8 = oh.astype(np.float32).astype(nf8)

    in_maps = []
    for c in range(NCORES):
        sl = slice(c * L, (c + 1) * L)
        in_maps.append({
            "txt_T": txt_T8,
            "that_T": that_T8,
            "that_T_loc": np.ascontiguousarray(that_T8[:, sl]),
            "ihat_T_loc": np.ascontiguousarray(
                ihat[sl].T.astype(np.float32)).astype(nf8),
            "oh_scaled": np.ascontiguousarray(
                (-BIG) * oh[:, sl].astype(np.float32)).astype(nf8),
            "oh_rhsT": oh_rhsT8,
        })

    # exact aligned-pair contribution sum_aligned (1 - cos), fp64 on host
    cnt = np.bincount(ins, minlength=G).astype(np.float64)
    IH = np.zeros((G, D))
    np.add.at(IH, ins, ihat)
    TH = np.zeros((G, D))
    np.add.at(TH, ins, that)
    corr = float((cnt ** 2).sum() - (IH * TH).sum())
    return in_maps, corr


def kernel(**inputs) -> np.ndarray:
    from concourse.bass_utils import run_bass_kernel_spmd

    if "nc" not in _CACHE:
        _CACHE["nc"] = _build_program()
    nc = _CACHE["nc"]
    in_maps, corr = _host_prep(**inputs)
    res = run_bass_kernel_spmd(nc, in_maps, core_ids=list(range(NCORES)),
                               trace=False)
    _CACHE["last_results"] = res
    total = np.float64(corr)
    for r in res.results:
        total += np.asarray(r["partials"], np.float64).sum()
    return np.float32(total / (N * N))


# revision 5
# speedup vs baseline: 3.7554x; 1.1724x over previous
# Trainium2 Bass kernel for nn_CustomImageCosineSimLoss (N=4096, D=512, 8 cores).
#
# Strategy (sharding_hint): shard image rows across the 8 cores (data parallel
# over i); text features / instruction ids replicated. Each core computes its
# [512, 4096] block of both pairwise matrices and 32 partial sums; the host
# adds the 8*32 partials plus a closed-form aligned-pair correction and
# divides by N^2 (the "all-reduce").
#
# Math per core (L=512 local rows, G=64 instruction groups):
#   device part = sum_ij relu(cos_ij - 8*mask_ij - w_ij)
# where sim'_ij = that_i . t_j (= sim_ij / n_i, so the min-max weights
# w_ij = (sim'_ij - mn'_i) * invr'_i match the reference up to an O(1e-7)
# epsilon shift), cos_ij = ihat_i . that_j, and the -8*mask one-hot matmul
# (folded into the cos PSUM group) forces relu() = 0 on aligned pairs.
# The exact aligned contribution sum_aligned (1 - cos) is computed on the
# host in fp64 from group sums (O(N*D) work).
#
# Device mapping per [128, 512] tile: PE does the sim'/cos matmuls in fp8
# DoubleRow mode (2x rate, fp32 PSUM) plus the plain-fp8 mask matmul; ACT
# copies sim' PSUM->SBUF (bf16); DVE does min/max stats, the fused
# x = pc - sim'*invr' pass, and relu(x + mn'*invr') with per-row
# accumulation. All operands arrive pre-normalized/transposed from the host
# (text is replicated, so normalization is O(N*D) host work), which keeps
# GPSIMD and the DMA-transpose path entirely out of the kernel.
import numpy as np
import ml_dtypes

import concourse.mybir as mybir
import concourse.tile as tile
from concourse import bacc
from concourse.bass import ts

BF16 = mybir.dt.bfloat16
F32 = mybir.dt.float32
FP8 = mybir.dt.float8e4
AF = mybir.ActivationFunctionType
OP = mybir.AluOpType
PM = mybir.MatmulPerfMode
nf8 = ml_dtypes.float8_e4m3

N, D, G, NCORES = 4096, 512, 64, 8
L = N // NCORES            # 512 local rows per core
KT = D // 128              # 4 contraction subtiles
KP = KT // 2               # 2 DoubleRow pairs
IT = L // 128              # 4 local i-tiles
JT = N // 512              # 8 j-blocks
BIG = 8.0
EPS_W = 1e-6

_CACHE = {}


def _build_program():
    nc = bacc.Bacc("TRN2", target_bir_lowering=False, debug=False,
                   enable_asserts=True, num_devices=NCORES)

    d_txt_T = nc.dram_tensor("txt_T", [D, N], FP8, kind="ExternalInput").ap()
    d_that_T = nc.dram_tensor("that_T", [D, N], FP8, kind="ExternalInput").ap()
    d_that_loc = nc.dram_tensor("that_T_loc", [D, L], FP8, kind="ExternalInput").ap()
    d_ihat_loc = nc.dram_tensor("ihat_T_loc", [D, L], FP8, kind="ExternalInput").ap()
    d_oh_scaled = nc.dram_tensor("oh_scaled", [G, L], FP8, kind="ExternalInput").ap()
    d_oh_rhsT = nc.dram_tensor("oh_rhsT", [G, N], FP8, kind="ExternalInput").ap()
    d_partials = nc.dram_tensor("partials", [128, IT * JT], F32,
                                kind="ExternalOutput").ap()

    with tile.TileContext(nc) as tc:
        with (
            tc.tile_pool(name="persist", bufs=1) as pp,
            tc.tile_pool(name="sims", bufs=IT) as psim,
            tc.tile_pool(name="x", bufs=3) as px,
            tc.tile_pool(name="junk", bufs=3) as pj,
            tc.tile_pool(name="stats", bufs=2) as pst,
            tc.tile_pool(name="psA", bufs=3, space="PSUM") as ppsA,
            tc.tile_pool(name="psB", bufs=3, space="PSUM") as ppsB,
        ):
            that_loc = pp.tile([128, KT * L], FP8)
            nc.sync.dma_start(that_loc[:].rearrange("p (c i) -> p c i", c=KT),
                              d_that_loc.rearrange("(c p) i -> p c i", p=128))
            that_loc_v = that_loc[:].rearrange("p (c i) -> p c i", c=KT)

            # txt_T loaded per j-block so the first sim' matmul starts early
            txt_T = pp.tile([128, KT * N], FP8)
            txt_T_v = txt_T[:].rearrange("p (c j) -> p c j", c=KT)
            d_txt_T_v = d_txt_T.rearrange("(c p) j -> p c j", p=128)
            for jt in range(JT):
                nc.sync.dma_start(txt_T_v[:, :, ts(jt, 512)],
                                  d_txt_T_v[:, :, ts(jt, 512)])

            ihat_loc = pp.tile([128, KT * L], FP8)
            nc.sync.dma_start(ihat_loc[:].rearrange("p (c i) -> p c i", c=KT),
                              d_ihat_loc.rearrange("(c p) i -> p c i", p=128))
            ihat_loc_v = ihat_loc[:].rearrange("p (c i) -> p c i", c=KT)
            oh_scaled = pp.tile([G, L], FP8)
            nc.sync.dma_start(oh_scaled[:], d_oh_scaled)
            oh_rhsT = pp.tile([G, N], FP8)
            nc.sync.dma_start(oh_rhsT[:], d_oh_rhsT)

            that_T = pp.tile([128, KT * N], FP8)
            that_T_v = that_T[:].rearrange("p (c j) -> p c j", c=KT)
            d_that_T_v = d_that_T.rearrange("(c p) j -> p c j", p=128)
            for jt in range(JT):
                nc.sync.dma_start(that_T_v[:, :, ts(jt, 512)],
                                  d_that_T_v[:, :, ts(jt, 512)])

            comb = pp.tile([128, IT * JT], F32)
            zeros = pp.tile([128, 512], BF16)
            nc.vector.memset(zeros[:], 0.0)
            sims, stats = {}, {}

            def emit_sim(it):
                sim_sb = psim.tile([128, N], BF16, tag="sim")
                for jt in range(JT):
                    ps = ppsA.tile([128, 512], F32, tag="mmA")
                    for kp in range(KP):
                        nc.tensor.matmul(ps[:],
                                         that_loc_v[:, 2 * kp:2 * kp + 2, ts(it, 128)],
                                         txt_T_v[:, 2 * kp:2 * kp + 2, ts(jt, 512)],
                                         start=(kp == 0), stop=(kp == KP - 1),
                                         perf_mode=PM.DoubleRow)
                    nc.scalar.copy(sim_sb[:, ts(jt, 512)], ps[:])
                sims[it] = sim_sb

            def emit_stats(it):
                sim_sb = sims[it]
                mn = pst.tile([128, 1], F32, tag="mn")
                nc.vector.tensor_reduce(out=mn[:], in_=sim_sb[:],
                                        axis=mybir.AxisListType.X, op=OP.min)
                mx = pst.tile([128, 1], F32, tag="mx")
                nc.vector.tensor_reduce(out=mx[:], in_=sim_sb[:],
                                        axis=mybir.AxisListType.X, op=OP.max)
                rng = pst.tile([128, 1], F32, tag="rng")
                nc.vector.tensor_tensor(out=rng[:], in0=mx[:], in1=mn[:],
                                        op=OP.subtract)
                nc.vector.tensor_scalar_add(out=rng[:], in0=rng[:], scalar1=EPS_W)
                invr = pst.tile([128, 1], F32, tag="invr")
                nc.vector.reciprocal(invr[:], rng[:])
                ninvr = pst.tile([128, 1], F32, tag="ninvr")
                nc.vector.tensor_scalar_mul(out=ninvr[:], in0=invr[:], scalar1=-1.0)
                mninvr = pst.tile([128, 1], F32, tag="mninvr")
                nc.vector.tensor_tensor(out=mninvr[:], in0=mn[:], in1=invr[:],
                                        op=OP.mult)
                stats[it] = (ninvr, mninvr)

            def emit_cos(it):
                sim_sb = sims[it]
                ninvr, mninvr = stats[it]
                for jt in range(JT):
                    pc = ppsB.tile([128, 512], F32, tag="mmB")
                    for kp in range(KP):
                        nc.tensor.matmul(pc[:],
                                         ihat_loc_v[:, 2 * kp:2 * kp + 2, ts(it, 128)],
                                         that_T_v[:, 2 * kp:2 * kp + 2, ts(jt, 512)],
                                         start=(kp == 0), stop=False,
                                         perf_mode=PM.DoubleRow)
                    nc.tensor.matmul(pc[:], oh_scaled[:, ts(it, 128)],
                                     oh_rhsT[:, ts(jt, 512)],
                                     start=False, stop=True)
                    # x = pc - sim' * invr'   (DVE, reads PSUM once)
                    x = px.tile([128, 512], BF16, tag="x")
                    nc.vector.scalar_tensor_tensor(
                        out=x[:], in0=sim_sb[:, ts(jt, 512)], scalar=ninvr[:],
                        in1=pc[:], op0=OP.mult, op1=OP.add)
                    # relu(x + mn'*invr') with per-row sum accumulation (DVE).
                    # NB: tensor_scalar's op1 becomes the REDUCE op when
                    # accum_out is set, so relu+sum needs the
                    # scalar_tensor_tensor form with an explicit zeros in1.
                    junk = pj.tile([128, 512], BF16, tag="junk")
                    nc.vector.scalar_tensor_tensor(
                        out=junk[:], in0=x[:], scalar=mninvr[:], in1=zeros[:],
                        op0=OP.add, op1=OP.max,
                        accum_out=comb[:, it * JT + jt:it * JT + jt + 1])

            # software pipeline: PE order sim0 sim1 cos0 sim2 cos1 sim3 cos2 cos3
            emit_sim(0)
            emit_stats(0)
            emit_sim(1)
            emit_stats(1)
            emit_cos(0)
            emit_sim(2)
            emit_stats(2)
            emit_cos(1)
            emit_sim(3)
            emit_stats(3)
            emit_cos(2)
            emit_cos(3)

            nc.sync.dma_start(d_partials, comb[:])

    nc.compile()
    return nc


def _host_prep(image_features, text_features, instr_d):
    img = np.asarray(image_features, np.float64)
    txt = np.asarray(text_features, np.float64)
    ins = np.asarray(instr_d).astype(np.int64)

    nt = np.linalg.norm(txt, axis=1)
    ni = np.linalg.norm(img, axis=1)
    that = txt / nt[:, None]
    ihat = img / ni[:, None]

    txt_T8 = np.ascontiguousarray(txt.T.astype(np.float32)).astype(nf8)
    that_T8 = np.ascontiguousarray(that.T.astype(np.float32)).astype(nf8)
    oh = ins[None, :] == np.arange(G, dtype=np.int64)[:, None]   # [G, N]
    oh_rhsT8 = oh.astype(np.float32).astype(nf8)

    in_maps = []
    for c in range(NCORES):
        sl = slice(c * L, (c + 1) * L)
        in_maps.append({
            "txt_T": txt_T8,
            "that_T": that_T8,
            "that_T_loc": np.ascontiguousarray(that_T8[:, sl]),
            "ihat_T_loc": np.ascontiguousarray(
                ihat[sl].T.astype(np.float32)).astype(nf8),
            "oh_scaled": np.ascontiguousarray(
                (-BIG) * oh[:, sl].astype(np.float32)).astype(nf8),
            "oh_rhsT": oh_rhsT8,
        })

    # exact aligned-pair contribution sum_aligned (1 - cos), fp64 on host
    cnt = np.bincount(ins, minlength=G).astype(np.float64)
    IH = np.zeros((G, D))
    np.add.at(IH, ins, ihat)
    TH = np.zeros((G, D))
    np.add.at(TH, ins, that)
    corr = float((cnt ** 2).sum() - (IH * TH).sum())
    return in_maps, corr


def kernel(**inputs) -> np.ndarray:
    from concourse.bass_utils import run_bass_kernel_spmd

    if "nc" not in _CACHE:
        _CACHE["nc"] = _build_program()
    nc = _CACHE["nc"]
    in_maps, corr = _host_prep(**inputs)
    res = run_bass_kernel_spmd(nc, in_maps, core_ids=list(range(NCORES)),
                               trace=False)
    _CACHE["last_results"] = res
    total = np.float64(corr)
    for r in res.results:
        total += np.asarray(r["partials"], np.float64).sum()
    return np.float32(total / (N * N))


# revision 6
# speedup vs baseline: 4.5600x; 1.2143x over previous
# Trainium2 Bass kernel for nn_CustomImageCosineSimLoss (N=4096, D=512, 8 cores).
#
# Strategy (sharding_hint): shard image rows across the 8 cores (data parallel
# over i); text features / instruction ids replicated. Each core computes its
# [512, 4096] block of both pairwise matrices and 32 relu partial sums plus
# per-row min-max stats; the host combines the partials with two closed-form
# corrections and divides by N^2 (the "all-reduce").
#
# Math per core (L=512 local rows):
#   device part = sum_ij relu(cos_ij - w_ij)        (over ALL pairs)
# with sim'_ij = that_i . t_j  (= sim_ij / n_i, so the min-max weights
# w_ij = (sim'_ij - mn'_i) * invr'_i match the reference up to an O(1e-7)
# epsilon shift) and cos_ij = ihat_i . that_j.  The host adds the exact
# aligned-pair term sum_aligned (1 - cos) (fp64 group sums, O(N*D)) and
# subtracts its own estimate of the aligned relu terms the device included,
# using the device-exported invr'/mn'*invr' stats (O(sum n_g^2 * D) work).
#
# Device mapping per [128, 512] tile: PE does the sim'/cos matmuls in fp8
# DoubleRow mode (fp32 PSUM); ACT copies sim' PSUM->SBUF (bf16) and does the
# final relu(x + mn'*invr') with per-row accumulation; DVE computes per-block
# min/max via tensor_scalar's op1-as-reduce accum form plus the fused
# x = pc - sim'*invr' pass.  All operands arrive pre-normalized/transposed
# from the host (text is replicated, so normalization is O(N*D) host work).
import numpy as np
import ml_dtypes

import concourse.mybir as mybir
import concourse.tile as tile
from concourse import bacc
from concourse.bass import ts

BF16 = mybir.dt.bfloat16
F32 = mybir.dt.float32
FP8 = mybir.dt.float8e4
AF = mybir.ActivationFunctionType
OP = mybir.AluOpType
PM = mybir.MatmulPerfMode
nf8 = ml_dtypes.float8_e4m3

N, D, G, NCORES = 4096, 512, 64, 8
L = N // NCORES            # 512 local rows per core
KT = D // 128              # 4 contraction subtiles
KP = KT // 2               # 2 DoubleRow pairs
IT = L // 128              # 4 local i-tiles
JT = N // 512              # 8 j-blocks
EPS_W = 1e-6

_CACHE = {}


def _build_program():
    nc = bacc.Bacc("TRN2", target_bir_lowering=False, debug=False,
                   enable_asserts=True, num_devices=NCORES)

    d_txt_T = nc.dram_tensor("txt_T", [D, N], FP8, kind="ExternalInput").ap()
    d_that_T = nc.dram_tensor("that_T", [D, N], FP8, kind="ExternalInput").ap()
    d_that_loc = nc.dram_tensor("that_T_loc", [D, L], FP8, kind="ExternalInput").ap()
    d_ihat_loc = nc.dram_tensor("ihat_T_loc", [D, L], FP8, kind="ExternalInput").ap()
    d_partials = nc.dram_tensor("partials", [128, IT * JT], F32,
                                kind="ExternalOutput").ap()
    d_stats = nc.dram_tensor("stats_out", [128, 2 * IT], F32,
                             kind="ExternalOutput").ap()

    with tile.TileContext(nc) as tc:
        with (
            tc.tile_pool(name="persist", bufs=1) as pp,
            tc.tile_pool(name="sims", bufs=IT) as psim,
            tc.tile_pool(name="x", bufs=3) as px,
            tc.tile_pool(name="junk", bufs=3) as pj,
            tc.tile_pool(name="stats", bufs=2) as pst,
            tc.tile_pool(name="psA", bufs=4, space="PSUM") as ppsA,
            tc.tile_pool(name="psB", bufs=4, space="PSUM") as ppsB,
        ):
            that_loc = pp.tile([128, KT * L], FP8)
            nc.sync.dma_start(that_loc[:].rearrange("p (c i) -> p c i", c=KT),
                              d_that_loc.rearrange("(c p) i -> p c i", p=128))
            that_loc_v = that_loc[:].rearrange("p (c i) -> p c i", c=KT)

            # txt_T loaded per j-block so the first sim' matmul starts early
            txt_T = pp.tile([128, KT * N], FP8)
            txt_T_v = txt_T[:].rearrange("p (c j) -> p c j", c=KT)
            d_txt_T_v = d_txt_T.rearrange("(c p) j -> p c j", p=128)
            for jt in range(JT):
                nc.sync.dma_start(txt_T_v[:, :, ts(jt, 512)],
                                  d_txt_T_v[:, :, ts(jt, 512)])

            ihat_loc = pp.tile([128, KT * L], FP8)
            nc.sync.dma_start(ihat_loc[:].rearrange("p (c i) -> p c i", c=KT),
                              d_ihat_loc.rearrange("(c p) i -> p c i", p=128))
            ihat_loc_v = ihat_loc[:].rearrange("p (c i) -> p c i", c=KT)

            that_T = pp.tile([128, KT * N], FP8)
            that_T_v = that_T[:].rearrange("p (c j) -> p c j", c=KT)
            d_that_T_v = d_that_T.rearrange("(c p) j -> p c j", p=128)
            for jt in range(JT):
                nc.sync.dma_start(that_T_v[:, :, ts(jt, 512)],
                                  d_that_T_v[:, :, ts(jt, 512)])

            comb = pp.tile([128, IT * JT], F32)
            stats_sb = pp.tile([128, 2 * IT], F32)   # invr / mninvr per it
            sims, stats = {}, {}

            def emit_sim(it):
                sim_sb = psim.tile([128, N], BF16, tag="sim")
                mn8 = pst.tile([128, JT], F32, tag="mn8")
                mx8 = pst.tile([128, JT], F32, tag="mx8")
                for jt in range(JT):
                    ps = ppsA.tile([128, 512], F32, tag="mmA")
                    for kp in range(KP):
                        nc.tensor.matmul(ps[:],
                                         that_loc_v[:, 2 * kp:2 * kp + 2, ts(it, 128)],
                                         txt_T_v[:, 2 * kp:2 * kp + 2, ts(jt, 512)],
                                         start=(kp == 0), stop=(kp == KP - 1),
                                         perf_mode=PM.DoubleRow)
                    nc.scalar.copy(sim_sb[:, ts(jt, 512)], ps[:])
                    # per-block row min/max: tensor_scalar with op1 as the
                    # REDUCE op (accum_out form); scalar2 is a no-op bound.
                    jk = pj.tile([128, 512], BF16, tag="junk")
                    nc.vector.tensor_scalar(
                        out=jk[:], in0=sim_sb[:, ts(jt, 512)], scalar1=0.0,
                        scalar2=1e30, op0=OP.add, op1=OP.min,
                        accum_out=mn8[:, jt:jt + 1])
                    jk2 = pj.tile([128, 512], BF16, tag="junk")
                    nc.vector.tensor_scalar(
                        out=jk2[:], in0=sim_sb[:, ts(jt, 512)], scalar1=0.0,
                        scalar2=-1e30, op0=OP.add, op1=OP.max,
                        accum_out=mx8[:, jt:jt + 1])
                sims[it] = sim_sb
                stats[it] = (mn8, mx8)

            def emit_stats(it):
                mn8, mx8 = stats[it]
                mn = pst.tile([128, 1], F32, tag="mn")
                jk = pj.tile([128, JT], F32, tag="junk8")
                nc.vector.tensor_scalar(
                    out=jk[:], in0=mn8[:], scalar1=0.0, scalar2=1e30,
                    op0=OP.add, op1=OP.min, accum_out=mn[:])
                mx = pst.tile([128, 1], F32, tag="mx")
                jk2 = pj.tile([128, JT], F32, tag="junk8")
                nc.vector.tensor_scalar(
                    out=jk2[:], in0=mx8[:], scalar1=0.0, scalar2=-1e30,
                    op0=OP.add, op1=OP.max, accum_out=mx[:])
                rng = pst.tile([128, 1], F32, tag="rng")
                nc.vector.tensor_tensor(out=rng[:], in0=mx[:], in1=mn[:],
                                        op=OP.subtract)
                nc.vector.tensor_scalar_add(out=rng[:], in0=rng[:], scalar1=EPS_W)
                invr = stats_sb[:, 2 * it:2 * it + 1]
                nc.vector.reciprocal(invr, rng[:])
                ninvr = pst.tile([128, 1], F32, tag="ninvr")
                nc.vector.tensor_scalar_mul(out=ninvr[:], in0=invr, scalar1=-1.0)
                mninvr = stats_sb[:, 2 * it + 1:2 * it + 2]
                nc.vector.tensor_tensor(out=mninvr, in0=mn[:], in1=invr,
                                        op=OP.mult)
                stats[it] = (ninvr, mninvr)

            def emit_cos(it):
                sim_sb = sims[it]
                ninvr, mninvr = stats[it]
                for jt in range(JT):
                    pc = ppsB.tile([128, 512], F32, tag="mmB")
                    for kp in range(KP):
                        nc.tensor.matmul(pc[:],
                                         ihat_loc_v[:, 2 * kp:2 * kp + 2, ts(it, 128)],
                                         that_T_v[:, 2 * kp:2 * kp + 2, ts(jt, 512)],
                                         start=(kp == 0), stop=(kp == KP - 1),
                                         perf_mode=PM.DoubleRow)
                    # x = pc - sim' * invr'   (DVE, reads PSUM once)
                    x = px.tile([128, 512], BF16, tag="x")
                    nc.vector.scalar_tensor_tensor(
                        out=x[:], in0=sim_sb[:, ts(jt, 512)], scalar=ninvr[:],
                        in1=pc[:], op0=OP.mult, op1=OP.add)
                    # relu(x + mn'*invr') with per-row sum accumulation (ACT)
                    jk = pj.tile([128, 512], BF16, tag="junk")
                    nc.scalar.activation(
                        out=jk[:], in_=x[:], func=AF.Relu, bias=mninvr,
                        scale=1.0,
                        accum_out=comb[:, it * JT + jt:it * JT + jt + 1])

            # software pipeline: PE order sim0 sim1 cos0 sim2 cos1 sim3 cos2 cos3
            emit_sim(0)
            emit_stats(0)
            emit_sim(1)
            emit_stats(1)
            emit_cos(0)
            emit_sim(2)
            emit_stats(2)
            emit_cos(1)
            emit_sim(3)
            emit_stats(3)
            emit_cos(2)
            emit_cos(3)

            nc.sync.dma_start(d_partials, comb[:])
            nc.sync.dma_start(d_stats, stats_sb[:])

    nc.compile()
    return nc


def _host_prep(image_features, text_features, instr_d):
    img = np.asarray(image_features, np.float64)
    txt = np.asarray(text_features, np.float64)
    ins = np.asarray(instr_d).astype(np.int64)

    nt = np.linalg.norm(txt, axis=1)
    ni = np.linalg.norm(img, axis=1)
    that = txt / nt[:, None]
    ihat = img / ni[:, None]

    txt_T8 = np.ascontiguousarray(txt.T.astype(np.float32)).astype(nf8)
    that_T8 = np.ascontiguousarray(that.T.astype(np.float32)).astype(nf8)

    in_maps = []
    for c in range(NCORES):
        sl = slice(c * L, (c + 1) * L)
        in_maps.append({
            "txt_T": txt_T8,
            "that_T": that_T8,
            "that_T_loc": np.ascontiguousarray(that_T8[:, sl]),
            "ihat_T_loc": np.ascontiguousarray(
                ihat[sl].T.astype(np.float32)).astype(nf8),
        })

    # exact aligned-pair contribution sum_aligned (1 - cos), fp64 on host
    cnt = np.bincount(ins, minlength=G).astype(np.float64)
    IH = np.zeros((G, D))
    np.add.at(IH, ins, ihat)
    TH = np.zeros((G, D))
    np.add.at(TH, ins, that)
    corr = float((cnt ** 2).sum() - (IH * TH).sum())
    return in_maps, corr, ins, txt, that, ihat


def _aligned_relu_sub(res, ins, txt, that, ihat):
    # Reconstruct per-row invr / mn*invr from the device stats dumps, then
    # estimate the aligned-pair relu terms the device summed (to subtract).
    invr = np.zeros(N)
    mninvr = np.zeros(N)
    for c, r in enumerate(res.results):
        st = np.asarray(r["stats_out"], np.float64)     # [128, 2*IT]
        for it in range(IT):
            rows = slice(c * L + it * 128, c * L + it * 128 + 128)
            invr[rows] = st[:, 2 * it]
            mninvr[rows] = st[:, 2 * it + 1]
    sub = 0.0
    for g in range(G):
        idx = np.where(ins == g)[0]
        if idx.size == 0:
            continue
        cosg = ihat[idx] @ that[idx].T
        simg = that[idx] @ txt[idx].T
        arg = cosg - simg * invr[idx][:, None] + mninvr[idx][:, None]
        sub += np.maximum(arg, 0.0).sum()
    return sub


def kernel(**inputs) -> np.ndarray:
    from concourse.bass_utils import run_bass_kernel_spmd

    if "nc" not in _CACHE:
        _CACHE["nc"] = _build_program()
    nc = _CACHE["nc"]
    in_maps, corr, ins, txt, that, ihat = _host_prep(**inputs)
    res = run_bass_kernel_spmd(nc, in_maps, core_ids=list(range(NCORES)),
                               trace=False)
    _CACHE["last_results"] = res
    total = np.float64(corr)
    for r in res.results:
        total += np.asarray(r["partials"], np.float64).sum()
    total -= _aligned_relu_sub(res, ins, txt, that, ihat)
    return np.float32(total / (N * N))


# revision 7
# speedup vs baseline: 4.8935x; 1.0731x over previous
# Trainium2 Bass kernel for nn_CustomImageCosineSimLoss (N=4096, D=512, 8 cores).
#
# Strategy (sharding_hint): shard image rows across the 8 cores (data parallel
# over i); text features / instruction ids replicated. Each core computes its
# [512, 4096] block of both pairwise matrices and 32 relu partial sums plus
# per-row min-max stats; the host combines the partials with two closed-form
# corrections and divides by N^2 (the "all-reduce").
#
# Math per core (L=512 local rows):
#   device part = sum_ij relu(cos_ij - w_ij)        (over ALL pairs)
# with sim'_ij = that_i . t_j  (= sim_ij / n_i, so the min-max weights
# w_ij = (sim'_ij - mn'_i) * invr'_i match the reference up to an O(1e-7)
# epsilon shift) and cos_ij = ihat_i . that_j.  The host adds the exact
# aligned-pair term sum_aligned (1 - cos) (fp64 group sums, O(N*D)) and
# subtracts its own estimate of the aligned relu terms the device included,
# using the device-exported invr'/mn'*invr' stats.
#
# Engine mapping per [128, 512] tile: PE does the sim'/cos matmuls in fp8
# DoubleRow mode (fp32 PSUM) and accumulates t = -sim'*invr' + mn'*invr'
# into the cos PSUM via an identity-stationary bf16 matmul, so ACT's relu
# (with sum accumulation) reads the finished relu argument straight from
# PSUM.  ACT also copies sim' PSUM->SBUF (bf16).  DVE only computes stats:
# half-row min/max reduces (low latency after each sim' sweep), the small
# scalar chain, and the per-block t tiles (one fused 2-scalar op each).
# All operands arrive pre-normalized/transposed/flat from the host (text is
# replicated, so normalization is O(N*D) host work); flat [128, x] DMA
# layouts keep descriptor generation cheap, and triggers alternate between
# the sync and gpsimd queues.
import numpy as np
import ml_dtypes

import concourse.mybir as mybir
import concourse.tile as tile
from concourse import bacc
from concourse.bass import ts

BF16 = mybir.dt.bfloat16
F32 = mybir.dt.float32
FP8 = mybir.dt.float8e4
AF = mybir.ActivationFunctionType
OP = mybir.AluOpType
PM = mybir.MatmulPerfMode
nf8 = ml_dtypes.float8_e4m3
nbf = ml_dtypes.bfloat16

N, D, G, NCORES = 4096, 512, 64, 8
L = N // NCORES            # 512 local rows per core
KT = D // 128              # 4 contraction subtiles
KP = KT // 2               # 2 DoubleRow pairs
IT = L // 128              # 4 local i-tiles
JT = N // 512              # 8 j-blocks
EPS_W = 1e-6

_CACHE = {}


def _build_program():
    nc = bacc.Bacc("TRN2", target_bir_lowering=False, debug=False,
                   enable_asserts=True, num_devices=NCORES)

    d_txtj = [nc.dram_tensor(f"txtj{j}", [128, KT * 512], FP8,
                             kind="ExternalInput").ap() for j in range(JT)]
    d_thatj = [nc.dram_tensor(f"thatj{j}", [128, KT * 512], FP8,
                              kind="ExternalInput").ap() for j in range(JT)]
    d_that_loc = nc.dram_tensor("that_loc", [128, KT * L], FP8,
                                kind="ExternalInput").ap()
    d_ihat_loc = nc.dram_tensor("ihat_loc", [128, KT * L], FP8,
                                kind="ExternalInput").ap()
    d_ident = nc.dram_tensor("ident", [128, 128], BF16,
                             kind="ExternalInput").ap()
    d_partials = nc.dram_tensor("partials", [128, IT * JT], F32,
                                kind="ExternalOutput").ap()
    d_stats = nc.dram_tensor("stats_out", [128, 2 * IT], F32,
                             kind="ExternalOutput").ap()

    with tile.TileContext(nc) as tc:
        with (
            tc.tile_pool(name="persist", bufs=1) as pp,
            tc.tile_pool(name="sims", bufs=3) as psim,
            tc.tile_pool(name="trow", bufs=3) as ptr,
            tc.tile_pool(name="junk", bufs=3) as pj,
            tc.tile_pool(name="stats", bufs=2) as pst,
            tc.tile_pool(name="psA", bufs=4, space="PSUM") as ppsA,
            tc.tile_pool(name="psB", bufs=4, space="PSUM") as ppsB,
        ):
            # operand loads: flat [128, x] host-prearranged layouts; first
            # sim' block only needs that_loc + txtj[0].
            that_loc = pp.tile([128, KT * L], FP8)
            nc.sync.dma_start(that_loc[:], d_that_loc)
            that_loc_v = that_loc[:].rearrange("p (c i) -> p c i", c=KT)

            txtj, thatj = [], []
            for j in range(JT):
                t_ = pp.tile([128, KT * 512], FP8, tag=f"txtj{j}")
                (nc.sync if j % 2 else nc.gpsimd).dma_start(t_[:], d_txtj[j])
                txtj.append(t_[:].rearrange("p (c j) -> p c j", c=KT))

            ihat_loc = pp.tile([128, KT * L], FP8)
            nc.gpsimd.dma_start(ihat_loc[:], d_ihat_loc)
            ihat_loc_v = ihat_loc[:].rearrange("p (c i) -> p c i", c=KT)
            ident = pp.tile([128, 128], BF16)
            nc.sync.dma_start(ident[:], d_ident)

            for j in range(JT):
                t_ = pp.tile([128, KT * 512], FP8, tag=f"thatj{j}")
                (nc.sync if j % 2 else nc.gpsimd).dma_start(t_[:], d_thatj[j])
                thatj.append(t_[:].rearrange("p (c j) -> p c j", c=KT))

            comb = pp.tile([128, IT * JT], F32)
            stats_sb = pp.tile([128, 2 * IT], F32)   # invr / mninvr per it
            sims, halves, stats = {}, {}, {}

            def emit_sim(it):
                sim_sb = psim.tile([128, N], BF16, tag="sim")
                mnH = pst.tile([128, 2], F32, tag="mnH")
                mxH = pst.tile([128, 2], F32, tag="mxH")
                for jt in range(JT):
                    ps = ppsA.tile([128, 512], F32, tag="mmA")
                    for kp in range(KP):
                        nc.tensor.matmul(ps[:],
                                         that_loc_v[:, 2 * kp:2 * kp + 2, ts(it, 128)],
                                         txtj[jt][:, 2 * kp:2 * kp + 2, :],
                                         start=(kp == 0), stop=(kp == KP - 1),
                                         perf_mode=PM.DoubleRow)
                    nc.scalar.copy(sim_sb[:, ts(jt, 512)], ps[:])
                    if jt in (3, 7):
                        h = jt // 4
                        nc.vector.tensor_reduce(
                            out=mnH[:, h:h + 1], in_=sim_sb[:, ts(h, 2048)],
                            axis=mybir.AxisListType.X, op=OP.min)
                        nc.vector.tensor_reduce(
                            out=mxH[:, h:h + 1], in_=sim_sb[:, ts(h, 2048)],
                            axis=mybir.AxisListType.X, op=OP.max)
                sims[it] = sim_sb
                halves[it] = (mnH, mxH)

            def emit_stats(it):
                mnH, mxH = halves[it]
                mn = pst.tile([128, 1], F32, tag="mn")
                nc.vector.tensor_reduce(out=mn[:], in_=mnH[:],
                                        axis=mybir.AxisListType.X, op=OP.min)
                mx = pst.tile([128, 1], F32, tag="mx")
                nc.vector.tensor_reduce(out=mx[:], in_=mxH[:],
                                        axis=mybir.AxisListType.X, op=OP.max)
                rng = pst.tile([128, 1], F32, tag="rng")
                nc.vector.tensor_tensor(out=rng[:], in0=mx[:], in1=mn[:],
                                        op=OP.subtract)
                nc.vector.tensor_scalar_add(out=rng[:], in0=rng[:], scalar1=EPS_W)
                invr = stats_sb[:, 2 * it:2 * it + 1]
                nc.vector.reciprocal(invr, rng[:])
                ninvr = pst.tile([128, 1], F32, tag="ninvr")
                nc.vector.tensor_scalar_mul(out=ninvr[:], in0=invr, scalar1=-1.0)
                mninvr = stats_sb[:, 2 * it + 1:2 * it + 2]
                nc.vector.tensor_tensor(out=mninvr, in0=mn[:], in1=invr,
                                        op=OP.mult)
                # t = -sim'*invr' + mn'*invr'  (bf16, one fused op per block)
                sim_sb = sims[it]
                t_row = ptr.tile([128, N], BF16, tag="t")
                for jt in range(JT):
                    nc.vector.tensor_scalar(
                        out=t_row[:, ts(jt, 512)], in0=sim_sb[:, ts(jt, 512)],
                        scalar1=ninvr[:], scalar2=mninvr,
                        op0=OP.mult, op1=OP.add)
                stats[it] = t_row

            def emit_cos(it):
                t_row = stats[it]
                for jt in range(JT):
                    pc = ppsB.tile([128, 512], F32, tag="mmB")
                    for kp in range(KP):
                        nc.tensor.matmul(pc[:],
                                         ihat_loc_v[:, 2 * kp:2 * kp + 2, ts(it, 128)],
                                         thatj[jt][:, 2 * kp:2 * kp + 2, :],
                                         start=(kp == 0), stop=False,
                                         perf_mode=PM.DoubleRow)
                    nc.tensor.matmul(pc[:], ident[:], t_row[:, ts(jt, 512)],
                                     start=False, stop=True)
                    # relu(pc) with per-row sum accumulation, straight off PSUM
                    jk = pj.tile([128, 512], BF16, tag="junk")
                    nc.scalar.activation(
                        out=jk[:], in_=pc[:], func=AF.Relu, bias=0.0, scale=1.0,
                        accum_out=comb[:, it * JT + jt:it * JT + jt + 1])

            # software pipeline (PE order): sim0 sim1 sim2 cos0 sim3 cos1 cos2 cos3
            emit_sim(0)
            emit_stats(0)
            emit_sim(1)
            emit_stats(1)
            emit_sim(2)
            emit_stats(2)
            emit_cos(0)
            emit_sim(3)
            emit_stats(3)
            emit_cos(1)
            emit_cos(2)
            emit_cos(3)

            nc.sync.dma_start(d_partials, comb[:])
            nc.sync.dma_start(d_stats, stats_sb[:])

    nc.compile()
    return nc


def _flat_dmajor(arr_T8, cols):
    # [D, cols] d-major -> flat SBUF layout [128, KT*cols]
    return np.ascontiguousarray(
        arr_T8.reshape(KT, 128, cols).transpose(1, 0, 2).reshape(128, KT * cols))


def _host_prep(image_features, text_features, instr_d):
    img = np.asarray(image_features, np.float64)
    txt = np.asarray(text_features, np.float64)
    ins = np.asarray(instr_d).astype(np.int64)

    nt = np.linalg.norm(txt, axis=1)
    ni = np.linalg.norm(img, axis=1)
    that = txt / nt[:, None]
    ihat = img / ni[:, None]

    txt_T8 = np.ascontiguousarray(txt.T.astype(np.float32)).astype(nf8)
    that_T8 = np.ascontiguousarray(that.T.astype(np.float32)).astype(nf8)
    ident = np.eye(128, dtype=nbf)

    shared = {}
    for j in range(JT):
        shared[f"txtj{j}"] = _flat_dmajor(
            np.ascontiguousarray(txt_T8[:, j * 512:(j + 1) * 512]), 512)
        shared[f"thatj{j}"] = _flat_dmajor(
            np.ascontiguousarray(that_T8[:, j * 512:(j + 1) * 512]), 512)
    shared["ident"] = ident

    in_maps = []
    for c in range(NCORES):
        sl = slice(c * L, (c + 1) * L)
        m = dict(shared)
        m["that_loc"] = _flat_dmajor(np.ascontiguousarray(that_T8[:, sl]), L)
        m["ihat_loc"] = _flat_dmajor(
            np.ascontiguousarray(ihat[sl].T.astype(np.float32)).astype(nf8), L)
        in_maps.append(m)

    # exact aligned-pair contribution sum_aligned (1 - cos), fp64 on host
    cnt = np.bincount(ins, minlength=G).astype(np.float64)
    IH = np.zeros((G, D))
    np.add.at(IH, ins, ihat)
    TH = np.zeros((G, D))
    np.add.at(TH, ins, that)
    corr = float((cnt ** 2).sum() - (IH * TH).sum())
    return in_maps, corr, ins, txt, that, ihat


def _aligned_relu_sub(res, ins, txt, that, ihat):
    # Reconstruct per-row invr / mn*invr from the device stats dumps, then
    # estimate the aligned-pair relu terms the device summed (to subtract).
    invr = np.zeros(N)
    mninvr = np.zeros(N)
    for c, r in enumerate(res.results):
        st = np.asarray(r["stats_out"], np.float64)     # [128, 2*IT]
        for it in range(IT):
            rows = slice(c * L + it * 128, c * L + it * 128 + 128)
            invr[rows] = st[:, 2 * it]
            mninvr[rows] = st[:, 2 * it + 1]
    sub = 0.0
    for g in range(G):
        idx = np.where(ins == g)[0]
        if idx.size == 0:
            continue
        cosg = ihat[idx] @ that[idx].T
        simg = that[idx] @ txt[idx].T
        arg = cosg - simg * invr[idx][:, None] + mninvr[idx][:, None]
        sub += np.maximum(arg, 0.0).sum()
    return sub


def kernel(**inputs) -> np.ndarray:
    from concourse.bass_utils import run_bass_kernel_spmd

    if "nc" not in _CACHE:
        _CACHE["nc"] = _build_program()
    nc = _CACHE["nc"]
    in_maps, corr, ins, txt, that, ihat = _host_prep(**inputs)
    res = run_bass_kernel_spmd(nc, in_maps, core_ids=list(range(NCORES)),
                               trace=False)
    _CACHE["last_results"] = res
    total = np.float64(corr)
    for r in res.results:
        total += np.asarray(r["partials"], np.float64).sum()
    total -= _aligned_relu_sub(res, ins, txt, that, ihat)
    return np.float32(total / (N * N))
